# revision 2
# baseline (speedup 1.0000x reference)
import os, sys, types, json

for _p in reversed(os.environ.get("NIX_PYTHONPATH", "").split(os.pathsep)):
    if _p and _p not in sys.path:
        sys.path.insert(0, _p)
if "/opt/trn_rl_repo" not in sys.path:
    sys.path.insert(0, "/opt/trn_rl_repo")

import numpy as np

N = 100000
E = 1600000
B = 64
F = 128
H = 64
A = 5
ROUTE_LEN = 10
EPS = 1e-5
NCORES = 8
NPC = N // NCORES          # 12500 nodes per core
NT = 104                   # col-tiles per core (static)
CPT = 17                   # chunks per tile
NCHUNK = NT * CPT          # 1768 chunks of 128 edge slots
DUMP = NPC                 # dump row for unused tile rows

LAST_RESULTS = []          # BassKernelResults per launch (for test harness)
_PROG = None


def _install_ntff_hook():
    try:
        import antenv.axon_hooks  # noqa: F401
        return
    except ImportError:
        pass
    try:
        import antenv
        mod = types.ModuleType("antenv.axon_hooks")
        _h = [None]
        mod.set_axon_ntff_profile_hook = lambda h: _h.__setitem__(0, h)
        mod.get_axon_ntff_profile_hook = lambda: _h[0]
        sys.modules["antenv.axon_hooks"] = mod
        antenv.axon_hooks = mod
        from trn_agent_boot.trn_boot import _ntff_profile_via_ctypes
        hook = _ntff_profile_via_ctypes("/opt/axon/libaxon_pjrt.so")
        if hook is not None:
            mod.set_axon_ntff_profile_hook(hook)
    except Exception:
        pass


def _split_multiwaits(nc, limit=1):
    """This walrus build allows only `limit` sem-wait per instruction; hoist
    extras onto preceding EventSemaphore instructions on the same engine."""
    orig = nc.to_json_bytes

    def patched():
        d = json.loads(orig())
        ctr = 0
        for f in d["functions"]:
            for bb in f["blocks"]:
                new = []
                for inst in bb["instructions"]:
                    si = inst.get("sync_info")
                    ow = (si or {}).get("on_wait") or []
                    if len(ow) > limit:
                        for w in ow[:-limit]:
                            ctr += 1
                            new.append({
                                "debug": inst.get("debug"),
                                "engine": inst["engine"],
                                "ins": [],
                                "outs": [],
                                "name": f"antsplitw_{ctr}",
                                "opcode": "EventSemaphore",
                                "sync_info": {"on_update": [], "on_wait": [w]},
                            })
                        si["on_wait"] = ow[-limit:]
                    new.append(inst)
                bb["instructions"] = new
        return json.dumps(d).encode()

    nc.to_json_bytes = patched


def _build_program():
    """One SPMD program: gather-aggregate one GCN layer for this core's
    12500-col shard. out_tiled[t, p, :] = sum_e S[e, tilecol p] * htab[row_e]."""
    global _PROG
    if _PROG is not None:
        return _PROG
    _install_ntff_hook()
    import concourse.bass as bass
    import concourse.mybir as mybir
    from concourse import tile
    from concourse.bass_utils import run_bass_kernel_spmd

    nc = bass.Bass()
    htab_d = nc.declare_dram_parameter("htab", [N, H], mybir.dt.bfloat16, isOutput=False)
    idx_d = nc.declare_dram_parameter("idx", [128, NCHUNK], mybir.dt.int32, isOutput=False)
    ctab_d = nc.declare_dram_parameter("ctab", [128, 2 * NCHUNK], mybir.dt.float32, isOutput=False)
    zout_d = nc.declare_dram_parameter("zout", [NT, 128, H], mybir.dt.float32, isOutput=True)

    with tile.TileContext(nc) as tc:
        with (
            tc.tile_pool(name="cst", bufs=1) as cst,
            tc.tile_pool(name="mp", bufs=6) as mp,
            tc.tile_pool(name="sp", bufs=6) as sp,
            tc.tile_pool(name="st", bufs=4) as st,
            tc.tile_pool(name="ps", bufs=4, space="PSUM") as ps,
        ):
            iota_i = cst.tile([128, 128], mybir.dt.int32)
            nc.gpsimd.iota(iota_i[:], pattern=[[1, 128]], base=0, channel_multiplier=0)
            iota_t = cst.tile([128, 128], mybir.dt.bfloat16)
            nc.vector.tensor_copy(iota_t[:], iota_i[:])
            idx_t = cst.tile([128, NCHUNK], mybir.dt.int32)
            nc.sync.dma_start(idx_t[:], idx_d[:])
            ctab_t = cst.tile([128, 2 * NCHUNK], mybir.dt.float32)
            nc.sync.dma_start(ctab_t[:], ctab_d[:])

            for t in range(NT):
                acc = ps.tile([128, H], mybir.dt.float32, space="PSUM")
                for j in range(CPT):
                    k = t * CPT + j
                    msg = mp.tile([128, H], mybir.dt.bfloat16)
                    nc.gpsimd.indirect_dma_start(
                        out=msg[:],
                        out_offset=None,
                        in_=htab_d[:],
                        in_offset=bass.IndirectOffsetOnAxis(
                            ap=idx_t[:, k : k + 1], axis=0
                        ),
                    )
                    s_t = sp.tile([128, 128], mybir.dt.bfloat16)
                    nc.vector.tensor_scalar(
                        out=s_t[:],
                        in0=iota_t[:],
                        scalar1=ctab_t[:, k : k + 1],
                        scalar2=ctab_t[:, NCHUNK + k : NCHUNK + k + 1],
                        op0=mybir.AluOpType.is_equal,
                        op1=mybir.AluOpType.mult,
                    )
                    nc.tensor.matmul(
                        acc[:], lhsT=s_t[:], rhs=msg[:],
                        start=(j == 0), stop=(j == CPT - 1),
                    )
                stage = st.tile([128, H], mybir.dt.float32)
                nc.vector.tensor_copy(stage[:], acc[:])
                nc.sync.dma_start(zout_d[t], stage[:])

    _split_multiwaits(nc)

    def launch(maps, trace=False):
        return run_bass_kernel_spmd(nc, maps, list(range(NCORES)), trace=trace)

    _PROG = launch
    return launch


def _pack_edges(row, col, norm):
    """Pack this core's edges (col already 0-based local, sorted by col) into
    NT tiles x CPT chunks x 128 slots. Returns idx[128,NCHUNK] int32,
    ctab[128,2*NCHUNK] f32, dest[NT,128] int32 (node index per tile row)."""
    order = np.argsort(col, kind="stable")
    row, col, norm = row[order], col[order], norm[order]
    ne = len(col)
    counts = np.bincount(col, minlength=NPC)

    cap = NT * CPT * 128
    idx_flat = np.zeros(cap, np.int64)
    coll_flat = np.zeros(cap, np.float32)
    norm_flat = np.zeros(cap, np.float32)
    dest = np.full((NT, 128), DUMP, np.int64)

    t = 0
    pos = 0        # next free slot in current tile (0..2176)
    c_start = 0    # first col of current tile
    e0 = 0         # edge cursor
    TSLOTS = CPT * 128
    for c in range(NPC):
        d = counts[c]
        if (pos + d > TSLOTS) or (c - c_start >= 128):
            t += 1
            pos = 0
            c_start = c
            if t >= NT:
                raise RuntimeError("tile overflow")
        base = t * TSLOTS + pos
        idx_flat[base : base + d] = row[e0 : e0 + d]
        coll_flat[base : base + d] = c - c_start
        norm_flat[base : base + d] = norm[e0 : e0 + d]
        w = c - c_start
        dest[t, w] = c
        pos += d
        e0 += d
    assert e0 == ne

    # chunk-major [p, k] layout: slot s of chunk k sits at [s, k]
    idx2 = idx_flat.reshape(NCHUNK, 128).T.astype(np.int32).copy()
    coll2 = coll_flat.reshape(NCHUNK, 128).T.copy()
    norm2 = norm_flat.reshape(NCHUNK, 128).T.copy()
    ctab = np.concatenate([coll2, norm2], axis=1)
    return idx2, np.ascontiguousarray(ctab), dest


def _prepare(edge_index, edge_weight):
    """Host preprocessing shared by both layers: per-core packed edge tables."""
    import ml_dtypes  # noqa: F401

    row = np.asarray(edge_index[0]).astype(np.int64)
    col = np.asarray(edge_index[1]).astype(np.int64)
    ew = np.asarray(edge_weight, np.float32)
    deg = np.bincount(col, weights=ew.astype(np.float64), minlength=N).astype(np.float32) + 1.0
    dis = 1.0 / np.sqrt(deg)

    # append self loops (weight 1)
    loop = np.arange(N, dtype=np.int64)
    rall = np.concatenate([row, loop])
    call = np.concatenate([col, loop])
    wall = np.concatenate([ew, np.ones(N, np.float32)])
    norm = dis[rall] * wall * dis[call]

    per_core = []
    cid = call // NPC
    for c in range(NCORES):
        m = cid == c
        idx2, ctab, dest = _pack_edges(rall[m], (call[m] - c * NPC), norm[m])
        per_core.append((idx2, ctab, dest))
    return per_core


def _aggregate_on_hw(h_full_f32, per_core, launch, trace=False):
    """One GCN aggregation layer on 8 cores. h_full [N, H] f32 -> z [N, H] f32."""
    import ml_dtypes

    htab = h_full_f32.astype(ml_dtypes.bfloat16)
    maps = [
        {"htab": htab, "idx": pc[0], "ctab": pc[1]}
        for pc in per_core
    ]
    res = launch(maps, trace=trace)
    LAST_RESULTS.append(res)
    z = np.zeros((N, H), np.float32)
    for c in range(NCORES):
        zt = np.asarray(res.results[c]["zout"])  # [NT, 128, H]
        dest = per_core[c][2]
        valid = dest < DUMP
        z[c * NPC + dest[valid]] = zt[valid]
    return z


def _bn(x, g, b):
    m = x.mean(0)
    v = x.var(0)
    return (x - m) / np.sqrt(v + EPS) * g + b


def _host_tail(h2, batch_idx, speed, route,
               sw, sb, sg, sbe, cw, cb, rg, rbe, rw, rb,
               ow1, ob1, og, obe, ow2, ob2):
    batch_idx = np.asarray(batch_idx).astype(np.int64)
    gx = np.full((B, H), -np.inf, np.float32)
    starts = np.searchsorted(batch_idx, np.arange(B), side="left")
    ends = np.searchsorted(batch_idx, np.arange(B), side="right")
    for bi in range(B):
        if ends[bi] > starts[bi]:
            gx[bi] = h2[starts[bi] : ends[bi]].max(0)

    v = np.maximum(_bn(np.asarray(speed) @ sw + sb, sg, sbe), 0.0)

    rt = np.asarray(route).transpose(0, 2, 1)
    rtp = np.pad(rt, ((0, 0), (0, 0), (1, 1)))
    rc = np.zeros((B, ROUTE_LEN), np.float32)
    for dt_ in range(3):
        rc += np.einsum("bit,i->bt", rtp[:, :, dt_ : dt_ + ROUTE_LEN], cw[0, :, dt_])
    rc = rc + cb[0]
    m = rc.mean()
    vv = rc.var()
    rc = (rc - m) / np.sqrt(vv + EPS) * rg[0] + rbe[0]
    rc = np.maximum(rc, 0.0)
    r = rc @ rw + rb

    cat = np.concatenate([gx, v.astype(np.float32), r.astype(np.float32)], axis=1)
    o = np.maximum(_bn(cat @ ow1 + ob1, og, obe), 0.0)
    o = o @ ow2 + ob2
    return np.squeeze(np.asarray(o, np.float32))


def _host_fallback(x, edge_index, edge_weight, b1, W1, g1, be1, W2, b2, g2, be2):
    """Pure-host aggregation path (correctness safety net)."""
    import scipy.sparse as sp

    row = np.asarray(edge_index[0]).astype(np.int64)
    col = np.asarray(edge_index[1]).astype(np.int64)
    ew = np.asarray(edge_weight, np.float32)
    deg = np.bincount(col, weights=ew.astype(np.float64), minlength=N).astype(np.float32) + 1.0
    dis = 1.0 / np.sqrt(deg)
    loop = np.arange(N, dtype=np.int64)
    rall = np.concatenate([row, loop])
    call = np.concatenate([col, loop])
    wall = np.concatenate([ew, np.ones(N, np.float32)])
    norm = dis[rall] * wall * dis[call]
    Amat = sp.csr_matrix((norm, (call, rall)), shape=(N, N))

    h = np.maximum(_bn(Amat @ (np.asarray(x) @ W1) + b1, g1, be1), 0.0)
    h2 = np.maximum(_bn(Amat @ (h @ W2) + b2, g2, be2), 0.0)
    return h2


def kernel(x, edge_index, edge_weight, batch_idx, speed, route,
           W1, b1, g1, be1, W2, b2, g2, be2,
           sw, sb, sg, sbe, cw, cb, rg, rbe, rw, rb,
           ow1, ob1, og, obe, ow2, ob2):
    x = np.asarray(x, np.float32)
    trace = bool(os.environ.get("GNN_TRACE"))
    try:
        launch = _build_program()
        per_core = _prepare(edge_index, edge_weight)
        h1 = x @ np.asarray(W1, np.float32)
        z1 = _aggregate_on_hw(h1, per_core, launch, trace=trace)
        zb1 = np.maximum(_bn(z1 + b1, g1, be1), 0.0)
        h2in = zb1 @ np.asarray(W2, np.float32)
        z2 = _aggregate_on_hw(h2in, per_core, launch, trace=trace)
        h2 = np.maximum(_bn(z2 + b2, g2, be2), 0.0)
    except Exception:
        import traceback
        traceback.print_exc()
        h2 = _host_fallback(x, edge_index, edge_weight, b1, W1, g1, be1, W2, b2, g2, be2)
    return _host_tail(h2, batch_idx, speed, route,
                      sw, sb, sg, sbe, cw, cb, rg, rbe, rw, rb,
                      ow1, ob1, og, obe, ow2, ob2)


# revision 3
# speedup vs baseline: 1.0419x; 1.0419x over previous
import os, sys, types, json

for _p in reversed(os.environ.get("NIX_PYTHONPATH", "").split(os.pathsep)):
    if _p and _p not in sys.path:
        sys.path.insert(0, _p)
if "/opt/trn_rl_repo" not in sys.path:
    sys.path.insert(0, "/opt/trn_rl_repo")

import numpy as np

N = 100000
E = 1600000
B = 64
F = 128
H = 64
A = 5
ROUTE_LEN = 10
EPS = 1e-5
NCORES = 8
NPC = N // NCORES          # 12500 nodes per core
NT = 100                   # col-tiles per core (static)
CPT = 17                   # chunks per tile
NCHUNK = NT * CPT          # 1768 chunks of 128 edge slots
DUMP = NPC                 # dump row for unused tile rows

LAST_RESULTS = []          # BassKernelResults per launch (for test harness)
_PROG = None


def _install_ntff_hook():
    try:
        import antenv.axon_hooks  # noqa: F401
        return
    except ImportError:
        pass
    try:
        import antenv
        mod = types.ModuleType("antenv.axon_hooks")
        _h = [None]
        mod.set_axon_ntff_profile_hook = lambda h: _h.__setitem__(0, h)
        mod.get_axon_ntff_profile_hook = lambda: _h[0]
        sys.modules["antenv.axon_hooks"] = mod
        antenv.axon_hooks = mod
        from trn_agent_boot.trn_boot import _ntff_profile_via_ctypes
        hook = _ntff_profile_via_ctypes("/opt/axon/libaxon_pjrt.so")
        if hook is not None:
            mod.set_axon_ntff_profile_hook(hook)
    except Exception:
        pass


def _split_multiwaits(nc, limit=1):
    """This walrus build allows only `limit` sem-wait per instruction; hoist
    extras onto preceding EventSemaphore instructions on the same engine."""
    orig = nc.to_json_bytes

    def patched():
        d = json.loads(orig())
        ctr = 0
        for f in d["functions"]:
            for bb in f["blocks"]:
                new = []
                for inst in bb["instructions"]:
                    si = inst.get("sync_info")
                    ow = (si or {}).get("on_wait") or []
                    if len(ow) > limit:
                        for w in ow[:-limit]:
                            ctr += 1
                            new.append({
                                "debug": inst.get("debug"),
                                "engine": inst["engine"],
                                "ins": [],
                                "outs": [],
                                "name": f"antsplitw_{ctr}",
                                "opcode": "EventSemaphore",
                                "sync_info": {"on_update": [], "on_wait": [w]},
                            })
                        si["on_wait"] = ow[-limit:]
                    new.append(inst)
                bb["instructions"] = new
        return json.dumps(d).encode()

    nc.to_json_bytes = patched


def _build_program():
    """One SPMD program: gather-aggregate one GCN layer for this core's
    12500-col shard. out_tiled[t, p, :] = sum_e S[e, tilecol p] * htab[row_e]."""
    global _PROG
    if _PROG is not None:
        return _PROG
    _install_ntff_hook()
    import concourse.bass as bass
    import concourse.mybir as mybir
    from concourse import tile
    from concourse.bass_utils import run_bass_kernel_spmd

    nc = bass.Bass()
    htab_d = nc.declare_dram_parameter("htab", [N, H], mybir.dt.bfloat16, isOutput=False)
    idx_d = nc.declare_dram_parameter("idx", [128, NCHUNK], mybir.dt.int32, isOutput=False)
    ctab_d = nc.declare_dram_parameter("ctab", [128, 2 * NCHUNK], mybir.dt.float32, isOutput=False)
    zout_d = nc.declare_dram_parameter("zout", [NT, 128, H], mybir.dt.float32, isOutput=True)

    with tile.TileContext(nc) as tc:
        with (
            tc.tile_pool(name="cst", bufs=1) as cst,
            tc.tile_pool(name="mp", bufs=16) as mp,
            tc.tile_pool(name="sp", bufs=16) as sp,
            tc.tile_pool(name="st", bufs=8) as st,
            tc.tile_pool(name="ps", bufs=4, space="PSUM") as ps,
        ):
            iota_i = cst.tile([128, 128], mybir.dt.int32)
            nc.gpsimd.iota(iota_i[:], pattern=[[1, 128]], base=0, channel_multiplier=0)
            iota_t = cst.tile([128, 128], mybir.dt.bfloat16)
            nc.vector.tensor_copy(iota_t[:], iota_i[:])
            idx_t = cst.tile([128, NCHUNK], mybir.dt.int32)
            nc.sync.dma_start(idx_t[:], idx_d[:])
            ctab_t = cst.tile([128, 2 * NCHUNK], mybir.dt.float32)
            nc.sync.dma_start(ctab_t[:], ctab_d[:])

            for t in range(NT):
                acc = ps.tile([128, H], mybir.dt.float32, space="PSUM")
                for j in range(CPT):
                    k = t * CPT + j
                    msg = mp.tile([128, H], mybir.dt.bfloat16)
                    nc.gpsimd.indirect_dma_start(
                        out=msg[:],
                        out_offset=None,
                        in_=htab_d[:],
                        in_offset=bass.IndirectOffsetOnAxis(
                            ap=idx_t[:, k : k + 1], axis=0
                        ),
                    )
                    s_t = sp.tile([128, 128], mybir.dt.bfloat16)
                    nc.vector.tensor_scalar(
                        out=s_t[:],
                        in0=iota_t[:],
                        scalar1=ctab_t[:, k : k + 1],
                        scalar2=ctab_t[:, NCHUNK + k : NCHUNK + k + 1],
                        op0=mybir.AluOpType.is_equal,
                        op1=mybir.AluOpType.mult,
                    )
                    nc.tensor.matmul(
                        acc[:], lhsT=s_t[:], rhs=msg[:],
                        start=(j == 0), stop=(j == CPT - 1),
                    )
                stage = st.tile([128, H], mybir.dt.float32)
                nc.vector.tensor_copy(stage[:], acc[:])
                nc.sync.dma_start(zout_d[t], stage[:])

    _split_multiwaits(nc)

    def launch(maps, trace=False):
        return run_bass_kernel_spmd(nc, maps, list(range(NCORES)), trace=trace)

    _PROG = launch
    return launch


def _pack_edges(row, col, norm):
    """Pack this core's edges (col already 0-based local, sorted by col) into
    NT tiles x CPT chunks x 128 slots. Returns idx[128,NCHUNK] int32,
    ctab[128,2*NCHUNK] f32, dest[NT,128] int32 (node index per tile row)."""
    order = np.argsort(col, kind="stable")
    row, col, norm = row[order], col[order], norm[order]
    ne = len(col)
    counts = np.bincount(col, minlength=NPC)

    cap = NT * CPT * 128
    idx_flat = np.zeros(cap, np.int64)
    coll_flat = np.zeros(cap, np.float32)
    norm_flat = np.zeros(cap, np.float32)
    dest = np.full((NT, 128), DUMP, np.int64)

    t = 0
    pos = 0        # next free slot in current tile (0..2176)
    c_start = 0    # first col of current tile
    e0 = 0         # edge cursor
    TSLOTS = CPT * 128
    for c in range(NPC):
        d = counts[c]
        if (pos + d > TSLOTS) or (c - c_start >= 128):
            t += 1
            pos = 0
            c_start = c
            if t >= NT:
                raise RuntimeError("tile overflow")
        base = t * TSLOTS + pos
        idx_flat[base : base + d] = row[e0 : e0 + d]
        coll_flat[base : base + d] = c - c_start
        norm_flat[base : base + d] = norm[e0 : e0 + d]
        w = c - c_start
        dest[t, w] = c
        pos += d
        e0 += d
    assert e0 == ne

    # chunk-major [p, k] layout: slot s of chunk k sits at [s, k]
    idx2 = idx_flat.reshape(NCHUNK, 128).T.astype(np.int32).copy()
    coll2 = coll_flat.reshape(NCHUNK, 128).T.copy()
    norm2 = norm_flat.reshape(NCHUNK, 128).T.copy()
    ctab = np.concatenate([coll2, norm2], axis=1)
    return idx2, np.ascontiguousarray(ctab), dest


def _prepare(edge_index, edge_weight):
    """Host preprocessing shared by both layers: per-core packed edge tables."""
    import ml_dtypes  # noqa: F401

    row = np.asarray(edge_index[0]).astype(np.int64)
    col = np.asarray(edge_index[1]).astype(np.int64)
    ew = np.asarray(edge_weight, np.float32)
    deg = np.bincount(col, weights=ew.astype(np.float64), minlength=N).astype(np.float32) + 1.0
    dis = 1.0 / np.sqrt(deg)

    # append self loops (weight 1)
    loop = np.arange(N, dtype=np.int64)
    rall = np.concatenate([row, loop])
    call = np.concatenate([col, loop])
    wall = np.concatenate([ew, np.ones(N, np.float32)])
    norm = dis[rall] * wall * dis[call]

    per_core = []
    cid = call // NPC
    for c in range(NCORES):
        m = cid == c
        idx2, ctab, dest = _pack_edges(rall[m], (call[m] - c * NPC), norm[m])
        per_core.append((idx2, ctab, dest))
    return per_core


def _aggregate_on_hw(h_full_f32, per_core, launch, trace=False):
    """One GCN aggregation layer on 8 cores. h_full [N, H] f32 -> z [N, H] f32."""
    import ml_dtypes

    htab = h_full_f32.astype(ml_dtypes.bfloat16)
    maps = [
        {"htab": htab, "idx": pc[0], "ctab": pc[1]}
        for pc in per_core
    ]
    res = launch(maps, trace=trace)
    LAST_RESULTS.append(res)
    z = np.zeros((N, H), np.float32)
    for c in range(NCORES):
        zt = np.asarray(res.results[c]["zout"])  # [NT, 128, H]
        dest = per_core[c][2]
        valid = dest < DUMP
        z[c * NPC + dest[valid]] = zt[valid]
    return z


def _bn(x, g, b):
    m = x.mean(0)
    v = x.var(0)
    return (x - m) / np.sqrt(v + EPS) * g + b


def _host_tail(h2, batch_idx, speed, route,
               sw, sb, sg, sbe, cw, cb, rg, rbe, rw, rb,
               ow1, ob1, og, obe, ow2, ob2):
    batch_idx = np.asarray(batch_idx).astype(np.int64)
    gx = np.full((B, H), -np.inf, np.float32)
    starts = np.searchsorted(batch_idx, np.arange(B), side="left")
    ends = np.searchsorted(batch_idx, np.arange(B), side="right")
    for bi in range(B):
        if ends[bi] > starts[bi]:
            gx[bi] = h2[starts[bi] : ends[bi]].max(0)

    v = np.maximum(_bn(np.asarray(speed) @ sw + sb, sg, sbe), 0.0)

    rt = np.asarray(route).transpose(0, 2, 1)
    rtp = np.pad(rt, ((0, 0), (0, 0), (1, 1)))
    rc = np.zeros((B, ROUTE_LEN), np.float32)
    for dt_ in range(3):
        rc += np.einsum("bit,i->bt", rtp[:, :, dt_ : dt_ + ROUTE_LEN], cw[0, :, dt_])
    rc = rc + cb[0]
    m = rc.mean()
    vv = rc.var()
    rc = (rc - m) / np.sqrt(vv + EPS) * rg[0] + rbe[0]
    rc = np.maximum(rc, 0.0)
    r = rc @ rw + rb

    cat = np.concatenate([gx, v.astype(np.float32), r.astype(np.float32)], axis=1)
    o = np.maximum(_bn(cat @ ow1 + ob1, og, obe), 0.0)
    o = o @ ow2 + ob2
    return np.squeeze(np.asarray(o, np.float32))


def _host_fallback(x, edge_index, edge_weight, b1, W1, g1, be1, W2, b2, g2, be2):
    """Pure-host aggregation path (correctness safety net)."""
    import scipy.sparse as sp

    row = np.asarray(edge_index[0]).astype(np.int64)
    col = np.asarray(edge_index[1]).astype(np.int64)
    ew = np.asarray(edge_weight, np.float32)
    deg = np.bincount(col, weights=ew.astype(np.float64), minlength=N).astype(np.float32) + 1.0
    dis = 1.0 / np.sqrt(deg)
    loop = np.arange(N, dtype=np.int64)
    rall = np.concatenate([row, loop])
    call = np.concatenate([col, loop])
    wall = np.concatenate([ew, np.ones(N, np.float32)])
    norm = dis[rall] * wall * dis[call]
    Amat = sp.csr_matrix((norm, (call, rall)), shape=(N, N))

    h = np.maximum(_bn(Amat @ (np.asarray(x) @ W1) + b1, g1, be1), 0.0)
    h2 = np.maximum(_bn(Amat @ (h @ W2) + b2, g2, be2), 0.0)
    return h2


def kernel(x, edge_index, edge_weight, batch_idx, speed, route,
           W1, b1, g1, be1, W2, b2, g2, be2,
           sw, sb, sg, sbe, cw, cb, rg, rbe, rw, rb,
           ow1, ob1, og, obe, ow2, ob2):
    x = np.asarray(x, np.float32)
    trace = bool(os.environ.get("GNN_TRACE"))
    try:
        launch = _build_program()
        per_core = _prepare(edge_index, edge_weight)
        h1 = x @ np.asarray(W1, np.float32)
        z1 = _aggregate_on_hw(h1, per_core, launch, trace=trace)
        zb1 = np.maximum(_bn(z1 + b1, g1, be1), 0.0)
        h2in = zb1 @ np.asarray(W2, np.float32)
        z2 = _aggregate_on_hw(h2in, per_core, launch, trace=trace)
        h2 = np.maximum(_bn(z2 + b2, g2, be2), 0.0)
    except Exception:
        import traceback
        traceback.print_exc()
        h2 = _host_fallback(x, edge_index, edge_weight, b1, W1, g1, be1, W2, b2, g2, be2)
    return _host_tail(h2, batch_idx, speed, route,
                      sw, sb, sg, sbe, cw, cb, rg, rbe, rw, rb,
                      ow1, ob1, og, obe, ow2, ob2)
